# revision 23
# baseline (speedup 1.0000x reference)
"""Bahdanau pointer-attention kernel for Trainium2 (8 NeuronCores, SPMD).

Computes energy[b, 1, n] = V . tanh(x[b, :N] @ W1.T + x[b, -1] @ W2.T)
for B=32, N=2048, D=1024.

Sharding: data-parallel over batch B across 8 cores (4 batches/core).
Per-core layout: contraction over d requires d on SBUF partitions, so the
host pre-transposes each core's x shard to [D, 4*N] during sharding.

Per-core pipeline (Tile framework):
  - keys matmul: psum[e128, n512] += W1T[d128, e128].T @ xT[d128, n512]
    (both operands float32r - 1 PE pass at full rate, ~fp22 precision)
  - ACT: tanh(psum + query_bias) fused via activation bias (per-partition)
  - V-dot: psum[1, n512] += VT[e128, 1].T @ tanh[e128, n512] on PE (bf16)
  - query preamble: psum[e128, b4] += W2T[d128, e128].T @ xqT[d128, b4]

Tuning notes (HW-measured via paired in-process differentials; per-session
device-time variance is +-40us, so cross-session numbers are untrustworthy):
  - keys MM stream (dma+mm only) runs ~268ns/MM = (128 w-load + 512
    stream)/2.4GHz; f32r reloads the stationary inline per matmul and no
    dedup exists (same-weights consecutive MMs measured SLOWER).
  - bf16 x/w: slower (~335ns/MM; separate LDWEIGHTS serializes).
  - fp8 DoubleRow: fails tolerance (rel err 0.030 vs 0.02 limit, numpy sim).
  - kpsum_bufs: 3 beats 4 (281 vs 331us head-to-head) and 5/6 (much worse).
  - vdot_pack (4x col-group concurrency): trips a flaky walrus compile bug
    once another module compiled in-process - unusable.
  - out stores via nc.scalar ring: serializes with ACT activations - worse.
  - vdot_dve=True (default): fold v and the ec-sum on the idle DVE
    (8 tensor_scalar_mul + 7 tensor_add per chunk), leaving ONE
    partition-reduce matmul per chunk on the PE instead of 8 bf16 MMs.
    Head-to-head same-session: 147us vs 335us - removing the bf16 vdot
    MMs from the f32r keys stream wins far more than their streaming
    cycles (PE dtype-switch overhead). rel err 3.5e-3 (bf16 DVE accum).
  - vdot_f32r (f32r final tree-add so the reduce-MM matches the keys
    dtype): same walrus in-process compile bug as vdot_pack - unusable.
  - kpsum_bufs=4 under vdot_dve: ties 3 (265 vs 268us). Keep 3.
"""

from contextlib import ExitStack

import numpy as np
import ml_dtypes

import concourse.bass as bass
import concourse.mybir as mybir
import concourse.tile as tile
from concourse import bacc
from concourse.bass_utils import run_bass_kernel_spmd

B, N, D = 32, 2048, 1024
CORES = 8
BPC = B // CORES            # batches per core
NTOT = BPC * N              # 8192 key positions per core
P = 128
DC = D // P                 # 8 d-chunks (contraction)
EC = D // P                 # 8 e-chunks (output feature)
NT = 512                    # n tile (one PSUM bank of f32)
NCH = NTOT // NT            # 16 n-chunks per core
NPB = N // NT               # n-chunks per batch

f32 = mybir.dt.float32
f32r = mybir.dt.float32r
bf16 = mybir.dt.bfloat16

TRACE = False
LAST_EXEC_NS = None
LAST_RESULTS = None

_NC_CACHE = {}


def _w_slice(w_sb, dc, ec):
    return w_sb[:, dc, ec * P:(ec + 1) * P]


def _body(ctx, tc, xT, xqT, w1T, w2T, vT, out, reps=1,
          do_dma=True, do_mm=True, do_act=True, do_vdot=True,
          split_dma=False, x_bufs=3, kpsum_bufs=3, group_n=1,
          same_w=False, xw_dt=f32r, w_dt=None, vdot_pack=False,
          act_bias=True, out_sq=False, vpsum_bufs=2, qpsum_bufs=2,
          vdot_dve=True, vdot_f32r=False):
    if w_dt is None:
        w_dt = xw_dt
    nc = tc.nc
    Tanh = mybir.ActivationFunctionType.Tanh

    w_pool = ctx.enter_context(tc.tile_pool(name="w", bufs=1))
    x_pool = ctx.enter_context(tc.tile_pool(name="x", bufs=x_bufs))
    t_pool = ctx.enter_context(
        tc.tile_pool(name="tanh", bufs=(2 * group_n + 1) * EC))
    small = ctx.enter_context(tc.tile_pool(name="small", bufs=1))
    en_pool = ctx.enter_context(tc.tile_pool(name="en", bufs=3))
    kpsum = ctx.enter_context(tc.tile_pool(name="kpsum", bufs=kpsum_bufs, space="PSUM"))
    vpsum = ctx.enter_context(tc.tile_pool(name="vpsum", bufs=vpsum_bufs, space="PSUM"))
    qpsum = ctx.enter_context(tc.tile_pool(name="qpsum", bufs=qpsum_bufs, space="PSUM"))

    # Resident weights, d-chunk on partitions: [p=128, (c, e)]
    w1_sb = w_pool.tile([P, DC, D], w_dt, tag="w1")
    nc.sync.dma_start(w1_sb[:], w1T.rearrange("(c p) e -> p c e", p=P))
    w2_sb = w_pool.tile([P, DC, D], w_dt, tag="w2")
    nc.sync.dma_start(w2_sb[:], w2T.rearrange("(c p) e -> p c e", p=P))
    v_sb = small.tile([P, EC], bf16, tag="v")
    nc.sync.dma_start(v_sb[:], vT[:, :])
    ones_sb = None
    vacc = None
    v32_sb = None
    vr_pool = None
    if vdot_dve:
        ones_sb = small.tile([P, 1], f32r if vdot_f32r else bf16, tag="ones")
        nc.vector.memset(ones_sb[:], 1.0)
        v32_sb = small.tile([P, EC], f32, tag="v32")
        nc.vector.tensor_copy(v32_sb[:], v_sb[:])  # DVE mult needs f32 scalar
        vacc = ctx.enter_context(tc.tile_pool(name="vacc", bufs=24))
        if vdot_f32r:
            # final tree-add writes f32r so the reduce-matmul stays in the
            # same PE dtype as the keys stream (no f32r<->bf16 switch)
            vr_pool = ctx.enter_context(tc.tile_pool(name="vr", bufs=3))
    xq_sb = small.tile([P, DC, BPC], xw_dt, tag="xq")
    nc.sync.dma_start(xq_sb[:], xqT.rearrange("(c p) b -> p c b", p=P))

    # Query preamble: q_sb[e128, (ec, b)] = x_query @ W2.T  (transposed)
    q_sb = small.tile([P, EC * BPC], f32, tag="q")
    for ec in range(EC):
        pq = qpsum.tile([P, BPC], f32)
        for dc in range(DC):
            nc.tensor.matmul(
                pq[:],
                lhsT=_w_slice(w2_sb, dc, ec),
                rhs=xq_sb[:, dc, :],
                start=(dc == 0),
                stop=(dc == DC - 1),
            )
        nc.vector.tensor_copy(q_sb[:, ec * BPC:(ec + 1) * BPC], pq[:])

    # Main loop, software-pipelined: V-dot for chunk k-1 is emitted after
    # the keys matmuls of chunk k so the PE never waits on ACT.
    pending = []  # [(tanh tiles, chunk index), ...]
    x_fixed = None
    GN = group_n
    if not do_dma:
        x_fixed = x_pool.tile([P, DC, NT], xw_dt, tag="x")
        nc.sync.dma_start(
            x_fixed[:], xT.rearrange("(c p) n -> p c n", p=P)[:, :, 0:NT])
    for rep_g in range(reps * NCH // GN):
        chs = [(rep_g * GN + j) % NCH for j in range(GN)]
        xs = []
        for ch in chs:
            if do_dma:
                x_sb = x_pool.tile([P, DC, NT], xw_dt, tag="x")
                src = xT.rearrange("(c p) n -> p c n", p=P)[
                    :, :, ch * NT:(ch + 1) * NT]
                if split_dma:
                    nc.sync.dma_start(x_sb[:, :DC // 2, :], src[:, :DC // 2, :])
                    nc.scalar.dma_start(x_sb[:, DC // 2:, :], src[:, DC // 2:, :])
                else:
                    nc.sync.dma_start(x_sb[:], src)
            else:
                x_sb = x_fixed
            xs.append(x_sb)
        if not do_mm:
            continue
        ttsl = [[] for _ in chs]
        for ec in range(EC):
            pks = [kpsum.tile([P, NT], f32, tag="pk", name=f"pk{g}")
                   for g in range(GN)]
            for dc in range(DC):
                for g in range(GN):
                    nc.tensor.matmul(
                        pks[g][:],
                        lhsT=_w_slice(w1_sb, 0 if same_w else dc, ec),
                        rhs=xs[g][:, dc, :],
                        start=(dc == 0),
                        stop=(dc == DC - 1),
                    )
            if not do_act:
                continue
            for g, ch in enumerate(chs):
                b = ch // NPB
                tt = t_pool.tile([P, NT], bf16, tag="tanh")
                bias = (q_sb[:, ec * BPC + b: ec * BPC + b + 1]
                        if act_bias else None)
                nc.scalar.activation(tt[:], pks[g][:], Tanh, bias=bias)
                ttsl[g].append(tt)
        if not (do_act and do_vdot):
            continue
        for p in pending:
            _emit_vdot(nc, vpsum, en_pool, v_sb, out, *p, pack=vdot_pack,
                       out_sq=out_sq, ones_sb=ones_sb, vacc=vacc,
                       v32_sb=v32_sb, vr_pool=vr_pool)
        pending = [(ttsl[g], chs[g]) for g in range(GN)]
    for p in pending:
        _emit_vdot(nc, vpsum, en_pool, v_sb, out, *p, pack=vdot_pack,
                   out_sq=out_sq, ones_sb=ones_sb, vacc=vacc,
                   v32_sb=v32_sb, vr_pool=vr_pool)


def _emit_vdot(nc, vpsum, en_pool, v_sb, out, tts, ch, pack=False,
               out_sq=False, ones_sb=None, vacc=None, v32_sb=None,
               vr_pool=None):
    out_eng = nc.scalar if out_sq else nc.sync
    if vacc is not None:
        # DVE path: fold v and the ec-sum on the (idle) vector engine, so
        # the PE does 1 partition-reduce matmul per chunk instead of 8.
        ms = []
        for ec in range(EC):
            m = vacc.tile([P, NT], bf16, tag="m")
            nc.vector.tensor_scalar_mul(m[:], tts[ec][:], v32_sb[:, ec:ec + 1])
            ms.append(m)
        while len(ms) > 1:
            nxt = []
            last = len(ms) == 2
            for a, b in zip(ms[0::2], ms[1::2]):
                if last and vr_pool is not None:
                    s = vr_pool.tile([P, NT], f32r, tag="vr")
                else:
                    s = vacc.tile([P, NT], bf16, tag="s")
                nc.vector.tensor_add(s[:], a[:], b[:])
                nxt.append(s)
            ms = nxt
        pv = vpsum.tile([1, NT], f32)
        nc.tensor.matmul(pv[:], lhsT=ones_sb[:], rhs=ms[0][:],
                         start=True, stop=True)
        en = en_pool.tile([1, NT], f32, tag="en")
        nc.vector.tensor_copy(en[:], pv[:])
        out_eng.dma_start(out[:, ch * NT:(ch + 1) * NT], en[:])
        return
    if not pack:
        pv = vpsum.tile([1, NT], f32)
        for ec in range(EC):
            nc.tensor.matmul(
                pv[:],
                lhsT=v_sb[:, ec:ec + 1],
                rhs=tts[ec][:],
                start=(ec == 0),
                stop=(ec == EC - 1),
            )
        en = en_pool.tile([1, NT], f32, tag="en")
        nc.vector.tensor_copy(en[:], pv[:])
        out_eng.dma_start(out[:, ch * NT:(ch + 1) * NT], en[:])
        return
    # Packed: 4 concurrent col-groups (output partitions 0/32/64/96),
    # each accumulating 2 e-chunks; DVE sums the 4 partial rows.
    pv = vpsum.tile([P, NT], f32, name="pvp", tag="pvp")
    for ec in range(EC):
        j = ec % 4
        nc.tensor.matmul(
            pv[32 * j:32 * j + 1, :],
            lhsT=v_sb[:, ec:ec + 1],
            rhs=tts[ec][:],
            start=(ec < 4),
            stop=(ec >= 4),
            tile_position=(0, 32 * j),
            skip_group_check=True,
        )
    s0 = en_pool.tile([1, NT], f32, name="s0", tag="s0")
    s1 = en_pool.tile([1, NT], f32, name="s1", tag="s1")
    nc.vector.tensor_add(s0[:], pv[0:1, :], pv[32:33, :])
    nc.vector.tensor_add(s1[:], pv[64:65, :], pv[96:97, :])
    en = en_pool.tile([1, NT], f32, tag="en")
    nc.vector.tensor_add(en[:], s0[:], s1[:])
    out_eng.dma_start(out[:, ch * NT:(ch + 1) * NT], en[:])


def build_module(reps=1, **opts):
    key = (reps, tuple(sorted(opts.items())))
    if key in _NC_CACHE:
        return _NC_CACHE[key]
    nc = bacc.Bacc("TRN2", target_bir_lowering=False, debug=False)
    xw_dt = opts.get("xw_dt", f32r)
    w_dt = opts.get("w_dt") or xw_dt
    xT = nc.declare_dram_parameter("xT", [D, NTOT], xw_dt, isOutput=False)
    xqT = nc.declare_dram_parameter("xqT", [D, BPC], xw_dt, isOutput=False)
    w1T = nc.declare_dram_parameter("w1T", [D, D], w_dt, isOutput=False)
    w2T = nc.declare_dram_parameter("w2T", [D, D], w_dt, isOutput=False)
    vT = nc.declare_dram_parameter("vT", [P, EC], bf16, isOutput=False)
    out = nc.declare_dram_parameter("out", [1, NTOT], f32, isOutput=True)
    with tile.TileContext(nc) as tc:
        with ExitStack() as ctx:
            _body(ctx, tc, xT, xqT, w1T, w2T, vT, out, reps=reps, **opts)
    nc.compile()
    _NC_CACHE[key] = nc
    return nc


def shard_inputs(x, W1, W2, V, xw_bf16=False, w_bf16=None):
    """Host-side sharding + layout transforms. Returns per-core input maps."""
    x = np.asarray(x, dtype=np.float32)
    bf = ml_dtypes.bfloat16
    xdt = bf if xw_bf16 else np.float32
    wdt = bf if (xw_bf16 if w_bf16 is None else w_bf16) else np.float32
    w1T = np.ascontiguousarray(np.asarray(W1, np.float32).T).astype(wdt)
    w2T = np.ascontiguousarray(np.asarray(W2, np.float32).T).astype(wdt)
    vT = np.ascontiguousarray(np.asarray(V, np.float32).reshape(EC, P).T).astype(bf)
    in_maps = []
    for c in range(CORES):
        xs = x[c * BPC:(c + 1) * BPC, :N, :]          # [BPC, N, D]
        xT = np.ascontiguousarray(xs.transpose(2, 0, 1)).reshape(D, NTOT).astype(xdt)
        xq = x[c * BPC:(c + 1) * BPC, N, :]           # [BPC, D]
        xqT = np.ascontiguousarray(xq.T).astype(xdt)  # [D, BPC]
        in_maps.append({
            "xT": xT, "xqT": xqT,
            "w1T": w1T, "w2T": w2T, "vT": vT,
        })
    return in_maps


def kernel(x, W1, W2, V, city_count):
    global LAST_EXEC_NS, LAST_RESULTS
    assert int(city_count) == N
    in_maps = shard_inputs(x, W1, W2, V)
    res = None
    # neuronx-cc has a flaky parallel-compile race (surfaces as a Python
    # exception) — rebuild and retry a couple of times if it hits.
    for attempt in range(3):
        try:
            nc = build_module()
            res = run_bass_kernel_spmd(nc, in_maps,
                                       core_ids=list(range(CORES)),
                                       trace=TRACE)
            break
        except Exception:
            _NC_CACHE.clear()
            if attempt == 2:
                raise
    LAST_EXEC_NS = res.exec_time_ns
    LAST_RESULTS = res
    out = np.concatenate(
        [res.results[c]["out"].reshape(BPC, N) for c in range(CORES)], axis=0
    )
    return out[:, None, :].astype(np.float32)



# revision 24
# speedup vs baseline: 1.0587x; 1.0587x over previous
"""Bahdanau pointer-attention kernel for Trainium2 (8 NeuronCores, SPMD).

Computes energy[b, 1, n] = V . tanh(x[b, :N] @ W1.T + x[b, -1] @ W2.T)
for B=32, N=2048, D=1024.

Sharding: data-parallel over batch B across 8 cores (4 batches/core).
Per-core layout: contraction over d requires d on SBUF partitions, so the
host pre-transposes each core's x shard to [D, 4*N] during sharding.

Per-core pipeline (Tile framework):
  - keys matmul: psum[e128, n512] += W1T[d128, e128].T @ xT[d128, n512]
    (both operands float32r - 1 PE pass at full rate, ~fp22 precision)
  - ACT: tanh(psum + query_bias) fused via activation bias (per-partition)
  - V-dot: psum[1, n512] += VT[e128, 1].T @ tanh[e128, n512] on PE (bf16)
  - query preamble: psum[e128, b4] += W2T[d128, e128].T @ xqT[d128, b4]

Tuning notes (HW-measured via paired in-process differentials; per-session
device-time variance is +-40us, so cross-session numbers are untrustworthy):
  - keys MM stream (dma+mm only) runs ~268ns/MM = (128 w-load + 512
    stream)/2.4GHz; f32r reloads the stationary inline per matmul and no
    dedup exists (same-weights consecutive MMs measured SLOWER).
  - bf16 x/w: slower (~335ns/MM; separate LDWEIGHTS serializes).
  - fp8 DoubleRow: fails tolerance (rel err 0.030 vs 0.02 limit, numpy sim).
  - kpsum_bufs: 3 beats 4 (281 vs 331us head-to-head) and 5/6 (much worse).
  - vdot_pack (4x col-group concurrency): trips a flaky walrus compile bug
    once another module compiled in-process - unusable.
  - out stores via nc.scalar ring: serializes with ACT activations - worse.
  - vdot_dve=True (default): fold v and the ec-sum on the idle DVE
    (8 tensor_scalar_mul + 7 tensor_add per chunk), leaving ONE
    partition-reduce matmul per chunk on the PE instead of 8 bf16 MMs.
    Head-to-head same-session: 147us vs 335us - removing the bf16 vdot
    MMs from the f32r keys stream wins far more than their streaming
    cycles (PE dtype-switch overhead). rel err 3.5e-3 (bf16 DVE accum).
  - vdot_f32r (f32r final tree-add so the reduce-MM matches the keys
    dtype): deterministically crashes walrus at NEFF compile (as do
    vdot_pack and bf16-lhsT/f32r-rhs mixing) - unusable. Note bass-side
    build_module() alone never invokes walrus; only a jit/NEFF compile
    proves a variant viable.
  - kpsum_bufs=4 under vdot_dve: ties 3 (265 vs 268us). Keep 3.
"""

from contextlib import ExitStack

import numpy as np
import ml_dtypes

import concourse.bass as bass
import concourse.mybir as mybir
import concourse.tile as tile
from concourse import bacc
from concourse.bass_utils import run_bass_kernel_spmd

B, N, D = 32, 2048, 1024
CORES = 8
BPC = B // CORES            # batches per core
NTOT = BPC * N              # 8192 key positions per core
P = 128
DC = D // P                 # 8 d-chunks (contraction)
EC = D // P                 # 8 e-chunks (output feature)
NT = 512                    # n tile (one PSUM bank of f32)
NCH = NTOT // NT            # 16 n-chunks per core
NPB = N // NT               # n-chunks per batch

f32 = mybir.dt.float32
f32r = mybir.dt.float32r
bf16 = mybir.dt.bfloat16

TRACE = False
LAST_EXEC_NS = None
LAST_RESULTS = None

_NC_CACHE = {}


def _w_slice(w_sb, dc, ec):
    return w_sb[:, dc, ec * P:(ec + 1) * P]


def _body(ctx, tc, xT, xqT, w1T, w2T, vT, out, reps=1,
          do_dma=True, do_mm=True, do_act=True, do_vdot=True,
          split_dma=False, x_bufs=3, kpsum_bufs=3, group_n=1,
          same_w=False, xw_dt=f32r, w_dt=None, vdot_pack=False,
          act_bias=True, out_sq=False, vpsum_bufs=2, qpsum_bufs=2,
          vdot_dve=True, vdot_f32r=False):
    if w_dt is None:
        w_dt = xw_dt
    nc = tc.nc
    Tanh = mybir.ActivationFunctionType.Tanh

    w_pool = ctx.enter_context(tc.tile_pool(name="w", bufs=1))
    x_pool = ctx.enter_context(tc.tile_pool(name="x", bufs=x_bufs))
    t_pool = ctx.enter_context(
        tc.tile_pool(name="tanh", bufs=(2 * group_n + 1) * EC))
    small = ctx.enter_context(tc.tile_pool(name="small", bufs=1))
    en_pool = ctx.enter_context(tc.tile_pool(name="en", bufs=3))
    kpsum = ctx.enter_context(tc.tile_pool(name="kpsum", bufs=kpsum_bufs, space="PSUM"))
    vpsum = ctx.enter_context(tc.tile_pool(name="vpsum", bufs=vpsum_bufs, space="PSUM"))
    qpsum = ctx.enter_context(tc.tile_pool(name="qpsum", bufs=qpsum_bufs, space="PSUM"))

    # Resident weights, d-chunk on partitions: [p=128, (c, e)]
    w1_sb = w_pool.tile([P, DC, D], w_dt, tag="w1")
    nc.sync.dma_start(w1_sb[:], w1T.rearrange("(c p) e -> p c e", p=P))
    w2_sb = w_pool.tile([P, DC, D], w_dt, tag="w2")
    nc.sync.dma_start(w2_sb[:], w2T.rearrange("(c p) e -> p c e", p=P))
    v_sb = small.tile([P, EC], bf16, tag="v")
    nc.sync.dma_start(v_sb[:], vT[:, :])
    ones_sb = None
    vacc = None
    v32_sb = None
    vr_pool = None
    if vdot_dve:
        ones_sb = small.tile([P, 1], f32r if vdot_f32r else bf16, tag="ones")
        nc.vector.memset(ones_sb[:], 1.0)
        v32_sb = small.tile([P, EC], f32, tag="v32")
        nc.vector.tensor_copy(v32_sb[:], v_sb[:])  # DVE mult needs f32 scalar
        vacc = ctx.enter_context(tc.tile_pool(name="vacc", bufs=24))
        if vdot_f32r:
            # final tree-add writes f32r so the reduce-matmul stays in the
            # same PE dtype as the keys stream (no f32r<->bf16 switch)
            vr_pool = ctx.enter_context(tc.tile_pool(name="vr", bufs=3))
    xq_sb = small.tile([P, DC, BPC], xw_dt, tag="xq")
    nc.sync.dma_start(xq_sb[:], xqT.rearrange("(c p) b -> p c b", p=P))

    # Query preamble: q_sb[e128, (ec, b)] = x_query @ W2.T  (transposed)
    q_sb = small.tile([P, EC * BPC], f32, tag="q")
    for ec in range(EC):
        pq = qpsum.tile([P, BPC], f32)
        for dc in range(DC):
            nc.tensor.matmul(
                pq[:],
                lhsT=_w_slice(w2_sb, dc, ec),
                rhs=xq_sb[:, dc, :],
                start=(dc == 0),
                stop=(dc == DC - 1),
            )
        nc.vector.tensor_copy(q_sb[:, ec * BPC:(ec + 1) * BPC], pq[:])

    # Main loop, software-pipelined: V-dot for chunk k-1 is emitted after
    # the keys matmuls of chunk k so the PE never waits on ACT.
    pending = []  # [(tanh tiles, chunk index), ...]
    x_fixed = None
    GN = group_n
    if not do_dma:
        x_fixed = x_pool.tile([P, DC, NT], xw_dt, tag="x")
        nc.sync.dma_start(
            x_fixed[:], xT.rearrange("(c p) n -> p c n", p=P)[:, :, 0:NT])
    for rep_g in range(reps * NCH // GN):
        chs = [(rep_g * GN + j) % NCH for j in range(GN)]
        xs = []
        for ch in chs:
            if do_dma:
                x_sb = x_pool.tile([P, DC, NT], xw_dt, tag="x")
                src = xT.rearrange("(c p) n -> p c n", p=P)[
                    :, :, ch * NT:(ch + 1) * NT]
                if split_dma:
                    nc.sync.dma_start(x_sb[:, :DC // 2, :], src[:, :DC // 2, :])
                    nc.scalar.dma_start(x_sb[:, DC // 2:, :], src[:, DC // 2:, :])
                else:
                    nc.sync.dma_start(x_sb[:], src)
            else:
                x_sb = x_fixed
            xs.append(x_sb)
        if not do_mm:
            continue
        ttsl = [[] for _ in chs]
        for ec in range(EC):
            pks = [kpsum.tile([P, NT], f32, tag="pk", name=f"pk{g}")
                   for g in range(GN)]
            for dc in range(DC):
                for g in range(GN):
                    nc.tensor.matmul(
                        pks[g][:],
                        lhsT=_w_slice(w1_sb, 0 if same_w else dc, ec),
                        rhs=xs[g][:, dc, :],
                        start=(dc == 0),
                        stop=(dc == DC - 1),
                    )
            if not do_act:
                continue
            for g, ch in enumerate(chs):
                b = ch // NPB
                tt = t_pool.tile([P, NT], bf16, tag="tanh")
                bias = (q_sb[:, ec * BPC + b: ec * BPC + b + 1]
                        if act_bias else None)
                nc.scalar.activation(tt[:], pks[g][:], Tanh, bias=bias)
                ttsl[g].append(tt)
        if not (do_act and do_vdot):
            continue
        for p in pending:
            _emit_vdot(nc, vpsum, en_pool, v_sb, out, *p, pack=vdot_pack,
                       out_sq=out_sq, ones_sb=ones_sb, vacc=vacc,
                       v32_sb=v32_sb, vr_pool=vr_pool)
        pending = [(ttsl[g], chs[g]) for g in range(GN)]
    for p in pending:
        _emit_vdot(nc, vpsum, en_pool, v_sb, out, *p, pack=vdot_pack,
                   out_sq=out_sq, ones_sb=ones_sb, vacc=vacc,
                   v32_sb=v32_sb, vr_pool=vr_pool)


def _emit_vdot(nc, vpsum, en_pool, v_sb, out, tts, ch, pack=False,
               out_sq=False, ones_sb=None, vacc=None, v32_sb=None,
               vr_pool=None):
    out_eng = nc.scalar if out_sq else nc.sync
    if vacc is not None:
        # DVE path: fold v and the ec-sum on the (idle) vector engine, so
        # the PE does 1 partition-reduce matmul per chunk instead of 8.
        ms = []
        for ec in range(EC):
            m = vacc.tile([P, NT], bf16, tag="m")
            nc.vector.tensor_scalar_mul(m[:], tts[ec][:], v32_sb[:, ec:ec + 1])
            ms.append(m)
        while len(ms) > 1:
            nxt = []
            last = len(ms) == 2
            for a, b in zip(ms[0::2], ms[1::2]):
                if last and vr_pool is not None:
                    s = vr_pool.tile([P, NT], f32r, tag="vr")
                else:
                    s = vacc.tile([P, NT], bf16, tag="s")
                nc.vector.tensor_add(s[:], a[:], b[:])
                nxt.append(s)
            ms = nxt
        pv = vpsum.tile([1, NT], f32)
        nc.tensor.matmul(pv[:], lhsT=ones_sb[:], rhs=ms[0][:],
                         start=True, stop=True)
        en = en_pool.tile([1, NT], f32, tag="en")
        nc.vector.tensor_copy(en[:], pv[:])
        out_eng.dma_start(out[:, ch * NT:(ch + 1) * NT], en[:])
        return
    if not pack:
        pv = vpsum.tile([1, NT], f32)
        for ec in range(EC):
            nc.tensor.matmul(
                pv[:],
                lhsT=v_sb[:, ec:ec + 1],
                rhs=tts[ec][:],
                start=(ec == 0),
                stop=(ec == EC - 1),
            )
        en = en_pool.tile([1, NT], f32, tag="en")
        nc.vector.tensor_copy(en[:], pv[:])
        out_eng.dma_start(out[:, ch * NT:(ch + 1) * NT], en[:])
        return
    # Packed: 4 concurrent col-groups (output partitions 0/32/64/96),
    # each accumulating 2 e-chunks; DVE sums the 4 partial rows.
    pv = vpsum.tile([P, NT], f32, name="pvp", tag="pvp")
    for ec in range(EC):
        j = ec % 4
        nc.tensor.matmul(
            pv[32 * j:32 * j + 1, :],
            lhsT=v_sb[:, ec:ec + 1],
            rhs=tts[ec][:],
            start=(ec < 4),
            stop=(ec >= 4),
            tile_position=(0, 32 * j),
            skip_group_check=True,
        )
    s0 = en_pool.tile([1, NT], f32, name="s0", tag="s0")
    s1 = en_pool.tile([1, NT], f32, name="s1", tag="s1")
    nc.vector.tensor_add(s0[:], pv[0:1, :], pv[32:33, :])
    nc.vector.tensor_add(s1[:], pv[64:65, :], pv[96:97, :])
    en = en_pool.tile([1, NT], f32, tag="en")
    nc.vector.tensor_add(en[:], s0[:], s1[:])
    out_eng.dma_start(out[:, ch * NT:(ch + 1) * NT], en[:])


def build_module(reps=1, **opts):
    key = (reps, tuple(sorted(opts.items())))
    if key in _NC_CACHE:
        return _NC_CACHE[key]
    nc = bacc.Bacc("TRN2", target_bir_lowering=False, debug=False)
    xw_dt = opts.get("xw_dt", f32r)
    w_dt = opts.get("w_dt") or xw_dt
    xT = nc.declare_dram_parameter("xT", [D, NTOT], xw_dt, isOutput=False)
    xqT = nc.declare_dram_parameter("xqT", [D, BPC], xw_dt, isOutput=False)
    w1T = nc.declare_dram_parameter("w1T", [D, D], w_dt, isOutput=False)
    w2T = nc.declare_dram_parameter("w2T", [D, D], w_dt, isOutput=False)
    vT = nc.declare_dram_parameter("vT", [P, EC], bf16, isOutput=False)
    out = nc.declare_dram_parameter("out", [1, NTOT], f32, isOutput=True)
    with tile.TileContext(nc) as tc:
        with ExitStack() as ctx:
            _body(ctx, tc, xT, xqT, w1T, w2T, vT, out, reps=reps, **opts)
    nc.compile()
    _NC_CACHE[key] = nc
    return nc


def shard_inputs(x, W1, W2, V, xw_bf16=False, w_bf16=None):
    """Host-side sharding + layout transforms. Returns per-core input maps."""
    x = np.asarray(x, dtype=np.float32)
    bf = ml_dtypes.bfloat16
    xdt = bf if xw_bf16 else np.float32
    wdt = bf if (xw_bf16 if w_bf16 is None else w_bf16) else np.float32
    w1T = np.ascontiguousarray(np.asarray(W1, np.float32).T).astype(wdt)
    w2T = np.ascontiguousarray(np.asarray(W2, np.float32).T).astype(wdt)
    vT = np.ascontiguousarray(np.asarray(V, np.float32).reshape(EC, P).T).astype(bf)
    in_maps = []
    for c in range(CORES):
        xs = x[c * BPC:(c + 1) * BPC, :N, :]          # [BPC, N, D]
        xT = np.ascontiguousarray(xs.transpose(2, 0, 1)).reshape(D, NTOT).astype(xdt)
        xq = x[c * BPC:(c + 1) * BPC, N, :]           # [BPC, D]
        xqT = np.ascontiguousarray(xq.T).astype(xdt)  # [D, BPC]
        in_maps.append({
            "xT": xT, "xqT": xqT,
            "w1T": w1T, "w2T": w2T, "vT": vT,
        })
    return in_maps


def kernel(x, W1, W2, V, city_count):
    global LAST_EXEC_NS, LAST_RESULTS
    assert int(city_count) == N
    in_maps = shard_inputs(x, W1, W2, V)
    res = None
    # neuronx-cc has a flaky parallel-compile race (surfaces as a Python
    # exception) — rebuild and retry a couple of times if it hits.
    for attempt in range(3):
        try:
            nc = build_module()
            res = run_bass_kernel_spmd(nc, in_maps,
                                       core_ids=list(range(CORES)),
                                       trace=TRACE)
            break
        except Exception:
            _NC_CACHE.clear()
            if attempt == 2:
                raise
    LAST_EXEC_NS = res.exec_time_ns
    LAST_RESULTS = res
    out = np.concatenate(
        [res.results[c]["out"].reshape(BPC, N) for c in range(CORES)], axis=0
    )
    return out[:, None, :].astype(np.float32)

